# revision 22
# baseline (speedup 1.0000x reference)
"""Trainium2 Bass kernel for a full transformer block (nn_Attention_32873679684330).

Sharding: data-parallel over batch — B=8 batch elements, one per NeuronCore.
Each core runs the full block (LN1 -> QKV -> attention -> out-proj+residual ->
LN2 -> GELU MLP -> residual) on its [1024, 1024] slice, fully on-chip.

v3 highlights over the bf16 baseline:
 - QKV, out-proj and attn@V matmuls run in fp8e4 with DoubleRow perf mode
   (2 contraction rows per PE pass). Softmax probs are produced in fp8
   directly by the Exp activation with an offset of -2 (cancelled by the
   self-consistent denominator from a ones-column appended to V).
 - Attention scores use PE row-tiling: head pairs occupy rows 0-63 /
   64-127 of the array concurrently (DH=64 contraction), doubling score
   throughput.
 - FFN stays bf16 (fp8 fails the 2e-2 tolerance) but weights are kept
   stationary for both token halves (2 matmuls per LDWEIGHTS).
 - Residual stream carried in fp32r; LN gammas/betas folded into weights
   and biases on the host. LN (x - mu) runs on the PE (rank-1 -mu + identity
   accumulate), rstd broadcast via GpSimd, so normalize is one DVE mul.
 - LN2 statistics are interleaved into the out-proj loop (per-chunk), V for
   heads 8-15 is produced inside the attention window to fill PE idle gaps.

Layouts: activations feature-major ([features(partitions), tokens(free)]).
LN stats via ones-vector matmuls.
"""

import sys, os

for _p in ("/root/.axon_site", "/root/.axon_site/_ro/trn_rl_repo",
           "/root/.axon_site/_ro/pypackages"):
    if _p not in sys.path:
        sys.path.append(_p)

import numpy as np
from contextlib import ExitStack

import concourse.bass as bass
import concourse.bacc as bacc
import concourse.mybir as mybir
import concourse.tile as tile
from concourse.bass_utils import run_bass_kernel_spmd

F32 = mybir.dt.float32
F32R = mybir.dt.float32r
BF16 = mybir.dt.bfloat16
FP8 = mybir.dt.float8e4
NP_BF16 = np.dtype(mybir.dt.np(BF16))
NP_FP8 = np.dtype(mybir.dt.np(FP8))
AF = mybir.ActivationFunctionType
ALU = mybir.AluOpType
DR = mybir.MatmulPerfMode.DoubleRow

B, P, E, H, DH, MLP = 8, 1024, 1024, 16, 64, 4096
SCALE = DH ** -0.5
NCORES = 8
EC = E // 128        # 8 feature chunks
TC = P // 128        # 8 token chunks
TN = P // 512        # 2 token halves
MC = MLP // 128      # 32 mlp chunks
HPAIRS = H // 2      # 8 head pairs
EXP_OFF = -2.0       # exp(s*SCALE - 2): keeps fp8 probs < 240 (self-cancelling)
VW = DH + 2          # v_sb row stride (64 v + 1 ones + 1 pad); slices use DH+1


def round_fp32r(x):
    b = np.ascontiguousarray(x, dtype=np.float32).view(np.uint32)
    b = ((b.astype(np.uint64) + 0x800) & 0xFFFFF000).astype(np.uint32)
    return b.view(np.float32)


def to_fp8(x):
    return np.clip(np.asarray(x, np.float32), -240.0, 240.0).astype(NP_FP8)


def build_program(dbg=False):
    nc = bacc.Bacc("TRN2", target_bir_lowering=False, debug=False,
                   num_devices=NCORES)

    xT_d = nc.dram_tensor("xT", [E, P], F32R, kind="ExternalInput").ap()
    wqkv8_d = nc.dram_tensor("wqkv8", [128, EC, 3 * E], FP8,
                             kind="ExternalInput").ap()
    wo8_d = nc.dram_tensor("wo8", [128, EC, E], FP8, kind="ExternalInput").ap()
    w1_d = nc.dram_tensor("w1", [128, MC, EC, 128], BF16,
                          kind="ExternalInput").ap()
    w2_d = nc.dram_tensor("w2", [128, MC, E], BF16, kind="ExternalInput").ap()
    bqkv_pm_d = nc.dram_tensor("bqkv_pm", [128, 16], F32,
                               kind="ExternalInput").ap()
    bv_row_d = nc.dram_tensor("bv_row", [1, E], F32R, kind="ExternalInput").ap()
    bo_pm_d = nc.dram_tensor("bo_pm", [128, EC], F32, kind="ExternalInput").ap()
    b1_pm_d = nc.dram_tensor("b1_pm", [128, MC], F32, kind="ExternalInput").ap()
    b2_pm_d = nc.dram_tensor("b2_pm", [128, EC], F32, kind="ExternalInput").ap()
    ones_row_d = nc.dram_tensor("ones_row", [1, 512], F32R,
                                kind="ExternalInput").ap()
    ones_col_d = nc.dram_tensor("ones_col", [128, 1], F32R,
                                kind="ExternalInput").ap()
    ident_d = nc.dram_tensor("ident", [128, 128], F32R,
                             kind="ExternalInput").ap()

    outT_d = nc.dram_tensor("outT", [E, P], F32, kind="ExternalOutput").ap()
    dbg_d = None
    if dbg:
        dbg_d = nc.dram_tensor("dbg", [8 * 1024, P], F32,
                               kind="ExternalOutput").ap()

    with tile.TileContext(nc) as tc, ExitStack() as ctx:
        const = ctx.enter_context(tc.tile_pool(name="const", bufs=1))
        rows = ctx.enter_context(tc.tile_pool(name="rows", bufs=2))
        scr = ctx.enter_context(tc.tile_pool(name="scr", bufs=2))

        def cload(shape, dt, dram, cname):
            t = const.tile(shape, dt, name=cname)
            nc.sync.dma_start(t[:], dram[:])
            return t

        ones_col = cload([128, 1], F32R, ones_col_d, "c_ones_col")
        ones_row = cload([1, 512], F32R, ones_row_d, "c_ones_row")
        ident = cload([128, 128], F32R, ident_d, "c_ident")
        ones_col_bf = const.tile([128, 1], BF16, name="c_ones_col_bf")
        nc.vector.memset(ones_col_bf[:], 1.0)

        # ---- load xT (f32r, feature-major) ----
        xTp = tc.alloc_tile_pool(name="xTp", bufs=1, side="right")
        xT = xTp.tile([128, EC, P], F32R, tag="xT", name="xT_sb")
        for c in range(EC):
            nc.sync.dma_start(xT[:, c, :], xT_d[c * 128:(c + 1) * 128, :])

        bqkv_pm = cload([128, 16], F32, bqkv_pm_d, "c_bqkv_pm")
        bv_row = cload([1, E], F32R, bv_row_d, "c_bv_row")
        bo_pm = cload([128, EC], F32, bo_pm_d, "c_bo_pm")
        b1_pm = cload([128, MC], F32, b1_pm_d, "c_b1_pm")
        b2_pm = cload([128, EC], F32, b2_pm_d, "c_b2_pm")
        eps_sb = const.tile([1, 1], F32, name="c_eps")
        nc.vector.memset(eps_sb[:], 1e-5)
        negoff = const.tile([128, 1], F32, name="c_negoff")
        nc.vector.memset(negoff[:], EXP_OFF)

        # ---- qkv / wo weights (fp8, fully resident) ----
        wo8p = tc.alloc_tile_pool(name="wo8p", bufs=1, side="right")
        wo8 = wo8p.tile([128, EC, E], FP8, tag="wo8", name="wo8_sb")
        nc.sync.dma_start(wo8[:], wo8_d[:])
        wq8p = tc.alloc_tile_pool(name="wq8p", bufs=1)
        wqkv8 = wq8p.tile([128, EC, 3 * E], FP8, tag="wqkv8", name="wqkv8_sb")
        for g in range(4):
            nc.sync.dma_start(wqkv8[:, 2 * g:2 * g + 2, :],
                              wqkv8_d[:, 2 * g:2 * g + 2, :])

        psA = tc.alloc_tile_pool(name="psA", bufs=4, space="PSUM")

        def ps_tile(nm):
            return psA.tile([128, 2, 512], F32, tag="ps", name=nm)

        # ---- LN helpers --------------------------------------------------
        def ln_rows(st, nm):
            """st: [128,2,512] psum tile, bank0 = sum(x), bank1 = sum(x^2).
            Returns (negmu_row f32r, rstd_row f32)."""
            negmu = rows.tile([1, 512], F32R, tag="mu", bufs=4,
                              name=f"{nm}_negmu")
            nc.scalar.activation(negmu[:], st[:1, 0, :], AF.Copy,
                                 scale=-1.0 / E)
            msq = rows.tile([1, 512], F32, tag="r", bufs=6, name=f"{nm}_msq")
            nc.scalar.activation(msq[:], st[:1, 1, :], AF.Copy, scale=1.0 / E)
            mu2 = rows.tile([1, 512], F32, tag="r", bufs=6, name=f"{nm}_mu2")
            nc.vector.tensor_mul(mu2[:], negmu[:], negmu[:])
            var = rows.tile([1, 512], F32, tag="r", bufs=6, name=f"{nm}_var")
            nc.vector.tensor_sub(var[:], msq[:], mu2[:])
            lv = rows.tile([1, 512], F32, tag="r", bufs=6, name=f"{nm}_lv")
            nc.scalar.activation(lv[:], var[:], AF.Ln, bias=eps_sb[:])
            rstd = rows.tile([1, 512], F32, tag="mu", bufs=4,
                             name=f"{nm}_rstd")
            nc.scalar.activation(rstd[:], lv[:], AF.Exp, scale=-0.5)
            return negmu, rstd

        def ln_normalize(src_t, out_t, out_nm, negmu_rows, rstd_rows):
            """out = (src - mu) * rstd via PE rank-1 + identity accumulate,
            GpSimd rstd broadcast, one DVE mul per chunk pair."""
            for tn in range(TN):
                sl = slice(tn * 512, (tn + 1) * 512)
                rb2 = scr.tile([128, 2, 512], F32, tag="rb2", bufs=2,
                               name=f"{out_nm}_rb{tn}")
                nc.gpsimd.partition_broadcast(rb2[:, 0, :], rstd_rows[tn][:])
                nc.gpsimd.partition_broadcast(rb2[:, 1, :], rstd_rows[tn][:])
                for cp in range(EC // 2):
                    dps = ps_tile(f"{out_nm}_d{tn}_{cp}")
                    for b2_ in range(2):
                        c = 2 * cp + b2_
                        nc.tensor.matmul(dps[:, b2_, :], ones_row[:, :128],
                                         negmu_rows[tn][:],
                                         start=True, stop=False)
                        nc.tensor.matmul(dps[:, b2_, :], ident[:],
                                         src_t[:, c, sl],
                                         start=False, stop=True)
                    nc.vector.tensor_mul(out_t[:, 2 * cp:2 * cp + 2, sl],
                                         dps[:], rb2[:])

        # ======== LN1 (gamma/beta folded into wqkv on host) ========
        xnp = tc.alloc_tile_pool(name="xnp", bufs=1)
        xnT = xnp.tile([128, EC, P], FP8, tag="xnT", name="xnT_sb")

        sq_tiles = []
        for c in range(EC):
            sq = scr.tile([128, P], BF16, tag="sq", bufs=8, name=f"ln1_sq{c}")
            nc.scalar.activation(sq[:], xT[:, c, :], AF.Square)
            sq_tiles.append(sq)

        ln1_negmu, ln1_rstd = [], []
        for tn in range(TN):
            sl = slice(tn * 512, (tn + 1) * 512)
            st = ps_tile(f"ln1_st{tn}")
            for c in range(EC):
                nc.tensor.matmul(st[:1, 0, :], ones_col[:], xT[:, c, sl],
                                 start=(c == 0), stop=(c == EC - 1))
            for c in range(EC):
                nc.tensor.matmul(st[:1, 1, :], ones_col_bf[:],
                                 sq_tiles[c][:, sl],
                                 start=(c == 0), stop=(c == EC - 1))
            negmu, rstd = ln_rows(st, f"ln1_{tn}")
            ln1_negmu.append(negmu)
            ln1_rstd.append(rstd)
        ln_normalize(xT, xnT, "xn1", ln1_negmu, ln1_rstd)

        def dump_fm(src_t, row0):
            dpool = tc.alloc_tile_pool(name=f"dump{row0}", bufs=1)
            for c in range(EC):
                st = dpool.tile([128, P], F32, tag="dump", name=f"dmp{row0}_{c}")
                nc.scalar.activation(st[:], src_t[:, c, :], AF.Copy)
                nc.sync.dma_start(dbg_d[row0 + c * 128:row0 + (c + 1) * 128, :],
                                  st[:])
            dpool.release()

        if dbg:
            dump_fm(xnT, 0)

        # ======== QKV ========
        # v: token-major output (xn stationary, weights moving)
        vp = tc.alloc_tile_pool(name="vp", bufs=1)
        v_sb = vp.tile([128, TC, H, VW], FP8, tag="v", name="v_sb")
        nc.vector.memset(v_sb[:, :, :, DH], 1.0)      # softmax-denominator col
        nc.vector.memset(v_sb[:, :, :, DH + 1], 0.0)  # pad

        def emit_v(vg):
            for tcp in range(TC // 2):
                ps = ps_tile(f"v_ps{vg}_{tcp}")
                for b_ in range(2):
                    tcc = 2 * tcp + b_
                    nc.tensor.matmul(ps[:, b_, :], ones_row[:, :128],
                                     bv_row[:, vg * 512:(vg + 1) * 512],
                                     start=True, stop=False)
                    for p8 in range(4):
                        nc.tensor.matmul(
                            ps[:, b_, :],
                            xnT[:, 2 * p8:2 * p8 + 2,
                                tcc * 128:(tcc + 1) * 128],
                            wqkv8[:, 2 * p8:2 * p8 + 2,
                                  2 * E + vg * 512:2 * E + (vg + 1) * 512],
                            start=False, stop=(p8 == 3), perf_mode=DR)
                for b_ in range(2):
                    tcc = 2 * tcp + b_
                    nc.vector.tensor_copy(
                        v_sb[:, tcc, vg * 8:(vg + 1) * 8, 0:DH],
                        ps[:, b_, :].rearrange("p (h d) -> p h d", d=DH))

        emit_v(0)

        # q, k then attention, per head pair
        qkp = tc.alloc_tile_pool(name="qkp", bufs=1)
        qT = qkp.tile([128, EC, P], BF16, tag="qT", name="qT_sb")
        kT = qkp.tile([128, EC, P], BF16, tag="kT", name="kT_sb")
        otp = tc.alloc_tile_pool(name="otp", bufs=1, side="right")
        oT = otp.tile([128, EC, P], FP8, tag="oT", name="oT_sb")
        atp = tc.alloc_tile_pool(name="atp", bufs=1, side="right")

        for hp in range(HPAIRS):
            for dst, fb, bi in ((qT, hp * 128, hp), (kT, E + hp * 128, 8 + hp)):
                ps = ps_tile(f"qk_ps{hp}_{fb}")
                for p8 in range(4):
                    lhsT = wqkv8[:, 2 * p8:2 * p8 + 2, fb:fb + 128]
                    for tn in range(TN):
                        nc.tensor.matmul(
                            ps[:, tn, :], lhsT,
                            xnT[:, 2 * p8:2 * p8 + 2,
                                tn * 512:(tn + 1) * 512],
                            start=(p8 == 0), stop=(p8 == 3), perf_mode=DR)
                nc.vector.tensor_scalar_add(
                    dst[:, hp, :], ps[:].rearrange("p a b -> p (a b)"),
                    bqkv_pm[:, bi:bi + 1])

            # ---- attention for heads (2hp, 2hp+1) ----
            for i in range(TN):
                isl = slice(i * 512, (i + 1) * 512)
                aT = atp.tile([128, 2, TC, 512], FP8, tag="aT", bufs=2,
                              name=f"aT_{hp}_{i}")
                for j in range(TC):
                    sps = ps_tile(f"s_ps{hp}_{i}_{j}")
                    jsl = slice(j * 128, (j + 1) * 128)
                    nc.tensor.matmul(sps[:, 0, :], kT[0:DH, hp, jsl],
                                     qT[0:DH, hp, isl], start=True, stop=True,
                                     tile_position=(0, 0))
                    nc.tensor.matmul(sps[:, 1, :], kT[DH:128, hp, jsl],
                                     qT[DH:128, hp, isl], start=True, stop=True,
                                     tile_position=(64, 0))
                    nc.scalar.activation(aT[:, :, j, :], sps[:], AF.Exp,
                                         scale=SCALE, bias=negoff[:])
                if dbg and hp == 0 and i == 0:
                    dap = tc.alloc_tile_pool(name="dumpa", bufs=1)
                    for jj in range(4):
                        sa = dap.tile([128, P], F32, tag="da",
                                      name=f"dmpa_{jj}")
                        for u2 in range(2):
                            nc.scalar.activation(
                                sa[:, u2 * 512:(u2 + 1) * 512],
                                aT[:, 0, 2 * jj + u2, :], AF.Copy)
                        nc.sync.dma_start(
                            dbg_d[7168 + jj * 128:7168 + (jj + 1) * 128, :],
                            sa[:])
                    dap.release()
                for u in range(2):
                    h = 2 * hp + u
                    pb = u * DH
                    ops = psA.tile([128, 512], F32, tag="ps",
                                   name=f"o_ps{hp}_{i}_{u}")
                    for jj in range(TC // 2):
                        nc.tensor.matmul(
                            ops[0:DH + 1, :],
                            v_sb[:, 2 * jj:2 * jj + 2, h, 0:DH + 1],
                            aT[:, u, 2 * jj:2 * jj + 2, :],
                            start=(jj == 0), stop=(jj == TC // 2 - 1),
                            perf_mode=DR)
                    den_row = rows.tile([1, 512], F32, tag="den", bufs=2,
                                        name=f"den_{hp}_{i}_{u}")
                    nc.vector.tensor_copy(den_row[:], ops[DH:DH + 1, :])
                    rec_row = rows.tile([1, 512], F32, tag="rec", bufs=2,
                                        name=f"rec_{hp}_{i}_{u}")
                    nc.vector.reciprocal_approx_fast(rec_row[:], den_row[:])
                    rec_b = scr.tile([DH, 512], F32, tag="recb", bufs=2,
                                     name=f"recb_{hp}_{i}_{u}")
                    nc.gpsimd.partition_broadcast(rec_b[:], rec_row[:])
                    nc.vector.tensor_mul(oT[pb:pb + DH, hp, isl],
                                         ops[0:DH, :], rec_b[:])

            if hp == 1:
                emit_v(1)   # fills PE idle in the exp-paced window

        if dbg:
            dump_fm(qT, 1024)
            dump_fm(kT, 2048)
            dvp = tc.alloc_tile_pool(name="dumpv", bufs=1)
            for tcc in range(TC):
                st = dvp.tile([128, H * DH], F32, tag="dumpv",
                              name=f"dmpv_{tcc}")
                nc.vector.tensor_copy(
                    st[:].rearrange("p (h d) -> p h d", d=DH),
                    v_sb[:, tcc, :, 0:DH])
                nc.sync.dma_start(
                    dbg_d[3072 + tcc * 128:3072 + (tcc + 1) * 128, :], st[:])
            dvp.release()
            dump_fm(oT, 4096)

        atp.release()
        qkp.release()
        vp.release()
        xnp.release()
        wq8p.release()

        # ======== out-proj + residual (fp8 DR), LN2 stats interleaved ======
        x2p = tc.alloc_tile_pool(name="x2p", bufs=1)
        x2T = x2p.tile([128, EC, P], F32R, tag="x2T", name="x2T_sb")
        st2 = [ps_tile(f"ln2_st{tn}") for tn in range(TN)]
        sq2_tiles = [scr.tile([128, P], BF16, tag="sq", bufs=8,
                              name=f"ln2_sq{c}") for c in range(EC)]
        for fc in range(EC):
            ps = ps_tile(f"wo_ps{fc}")
            for p8 in range(4):
                lhsT = wo8[:, 2 * p8:2 * p8 + 2, fc * 128:(fc + 1) * 128]
                for tn in range(TN):
                    nc.tensor.matmul(
                        ps[:, tn, :], lhsT,
                        oT[:, 2 * p8:2 * p8 + 2, tn * 512:(tn + 1) * 512],
                        start=(p8 == 0), stop=(p8 == 3), perf_mode=DR)
            nc.vector.scalar_tensor_tensor(
                x2T[:, fc, :], ps[:].rearrange("p a b -> p (a b)"),
                bo_pm[:, fc:fc + 1], xT[:, fc, :], op0=ALU.add, op1=ALU.add)
            # LN2 stats for this chunk (accumulate across the wo loop)
            nc.scalar.activation(sq2_tiles[fc][:], x2T[:, fc, :], AF.Square)
            for tn in range(TN):
                sl = slice(tn * 512, (tn + 1) * 512)
                nc.tensor.matmul(st2[tn][:1, 0, :], ones_col[:],
                                 x2T[:, fc, sl],
                                 start=(fc == 0), stop=(fc == EC - 1))
                nc.tensor.matmul(st2[tn][:1, 1, :], ones_col_bf[:],
                                 sq2_tiles[fc][:, sl],
                                 start=(fc == 0), stop=(fc == EC - 1))

        if dbg:
            dump_fm(x2T, 5120)

        otp.release()
        wo8p.release()
        xTp.release()

        # ======== LN2 rows + normalize (gamma/beta folded into w1/b1) ======
        xn2p = tc.alloc_tile_pool(name="xn2p", bufs=1)
        xn2T = xn2p.tile([128, EC, P], BF16, tag="xn2T", name="xn2T_sb")
        ln2_negmu, ln2_rstd = [], []
        for tn in range(TN):
            negmu, rstd = ln_rows(st2[tn], f"ln2_{tn}")
            ln2_negmu.append(negmu)
            ln2_rstd.append(rstd)
        ln_normalize(x2T, xn2T, "xn2", ln2_negmu, ln2_rstd)

        if dbg:
            dump_fm(xn2T, 6144)

        psA.release()

        # ======== FFN1 (bf16; 2 matmuls per LDWEIGHTS via token halves) ====
        w1p = tc.alloc_tile_pool(name="w1p", bufs=4)
        hp_pool = tc.alloc_tile_pool(name="hp", bufs=1, side="right")
        hT = hp_pool.tile([128, MC, P], BF16, tag="hT", name="hT_sb")
        psF = tc.alloc_tile_pool(name="psF", bufs=2, space="PSUM")

        for mc in range(MC):
            w1t = w1p.tile([128, EC, 128], BF16, tag="w1t", name=f"w1t_{mc}")
            nc.sync.dma_start(w1t[:], w1_d[:, mc, :, :])
            ps = psF.tile([128, 2, 512], F32, tag="f", name=f"h_ps{mc}")
            for ec in range(EC):
                lhsT = w1t[:, ec, :]
                for tn in range(TN):
                    nc.tensor.matmul(ps[:, tn, :], lhsT,
                                     xn2T[:, ec, tn * 512:(tn + 1) * 512],
                                     start=(ec == 0), stop=(ec == EC - 1))
            nc.scalar.activation(hT[:, mc, :],
                                 ps[:].rearrange("p a b -> p (a b)"),
                                 AF.Gelu, bias=b1_pm[:, mc:mc + 1])

        psF.release()

        # ======== FFN2 (bf16; 4 output chunks accumulate per fg pass) ======
        w1p.release()
        xn2p.release()
        w2p = tc.alloc_tile_pool(name="w2p", bufs=8)
        ogp = tc.alloc_tile_pool(name="ogp", bufs=2)
        psG = tc.alloc_tile_pool(name="psG", bufs=4, space="PSUM")

        for fg in range(2):
            ps2 = [psG.tile([128, 2, 512], F32, tag="g", name=f"o_ps{fg}_{fc}")
                   for fc in range(4)]
            for mc in range(MC):
                w2t = w2p.tile([128, 512], BF16, tag="w2t",
                               name=f"w2t_{fg}_{mc}")
                nc.sync.dma_start(w2t[:],
                                  w2_d[:, mc, fg * 512:(fg + 1) * 512])
                for fc in range(4):
                    lhsT = w2t[:, fc * 128:(fc + 1) * 128]
                    for tn in range(TN):
                        nc.tensor.matmul(
                            ps2[fc][:, tn, :], lhsT,
                            hT[:, mc, tn * 512:(tn + 1) * 512],
                            start=(mc == 0), stop=(mc == MC - 1))
            for fc in range(4):
                f = fg * 4 + fc
                og = ogp.tile([128, P], F32, tag="og", name=f"og_{f}")
                nc.vector.scalar_tensor_tensor(
                    og[:], ps2[fc][:].rearrange("p a b -> p (a b)"),
                    b2_pm[:, f:f + 1], x2T[:, f, :], op0=ALU.add, op1=ALU.add)
                nc.sync.dma_start(outT_d[f * 128:(f + 1) * 128, :], og[:])

        ogp.release()
        w2p.release()
        x2p.release()
        hp_pool.release()
        psG.release()
    return nc


def prep_inputs(x, ln1_g, ln1_b, wqkv, bqkv, wo, bo, ln2_g, ln2_b, w1, b1,
                w2, b2):
    """Host-side prep: shard x over batch, transpose feature-major, fold LN
    gammas/betas into the adjacent weights, quantize/cast, reorder weight
    blocks for chunked stationary loads."""
    ln1_g = np.asarray(ln1_g, np.float32)
    ln1_b = np.asarray(ln1_b, np.float32)
    ln2_g = np.asarray(ln2_g, np.float32)
    ln2_b = np.asarray(ln2_b, np.float32)
    wqkv = np.asarray(wqkv, np.float32)
    bqkv = np.asarray(bqkv, np.float32)
    wo_ = np.asarray(wo, np.float32)
    w1_ = np.asarray(w1, np.float32)
    w2_ = np.asarray(w2, np.float32)
    b1_ = np.asarray(b1, np.float32)

    # fold LN1 gamma into wqkv rows, beta into bqkv
    wqkv_f = wqkv * ln1_g[:, None]
    bqkv_f = bqkv + ln1_b @ wqkv
    # fold LN2 gamma into w1 rows, beta into b1
    w1_f = w1_ * ln2_g[:, None]
    b1_f = b1_ + ln2_b @ w1_

    def blk(w):  # [E, F] -> [128, EC_rows, F]
        Erows = w.shape[0]
        return np.ascontiguousarray(
            w.reshape(Erows // 128, 128, -1).transpose(1, 0, 2))

    def pm(vec, nchunks):
        return np.ascontiguousarray(
            np.asarray(vec, np.float32).reshape(nchunks, 128).T)

    w1_blk = blk(w1_f)                                  # [128, 8, 4096]
    w1_r = np.ascontiguousarray(                        # [128, 32(mc), 8, 128]
        w1_blk.reshape(128, EC, MC, 128).transpose(0, 2, 1, 3))

    shared = dict(
        wqkv8=to_fp8(blk(wqkv_f)),
        wo8=to_fp8(blk(wo_)),
        w1=w1_r.astype(NP_BF16),
        w2=blk(w2_).astype(NP_BF16),                    # [128, 32(mc), 1024]
        bqkv_pm=pm(bqkv_f[:2 * E], 16),
        bv_row=round_fp32r(bqkv_f[2 * E:].reshape(1, E)),
        bo_pm=pm(np.asarray(bo, np.float32), EC),
        b1_pm=pm(b1_f, MC),
        b2_pm=pm(np.asarray(b2, np.float32), EC),
        ones_row=np.ones((1, 512), np.float32),
        ones_col=np.ones((128, 1), np.float32),
        ident=np.eye(128, dtype=np.float32),
    )
    x = np.asarray(x, np.float32)
    in_maps = []
    for b in range(B):
        m = dict(shared)
        m["xT"] = round_fp32r(np.ascontiguousarray(x[b, :, :E].T))
        in_maps.append(m)
    return in_maps


_CACHE = {}


def run_on_hw(inputs, stage="full", trace=False, **trace_kw):
    key = stage
    if key not in _CACHE:
        nc = build_program(dbg=(stage == "dbg"))
        nc.compile()
        _CACHE[key] = nc
    nc = _CACHE[key]
    in_maps = prep_inputs(**inputs)
    res = run_bass_kernel_spmd(nc, in_maps, list(range(NCORES)), trace=trace,
                               **trace_kw)
    return res


def kernel(**inputs) -> np.ndarray:
    res = run_on_hw(inputs, stage="full", trace=False)
    out = np.zeros((B, P, E + 1), np.float32)
    for b in range(B):
        out[b, :, :E] = res.results[b]["outT"].T
    return out


# revision 24
# speedup vs baseline: 1.0158x; 1.0158x over previous
"""Trainium2 Bass kernel for a full transformer block (nn_Attention_32873679684330).

Sharding: data-parallel over batch — B=8 batch elements, one per NeuronCore.
Each core runs the full block (LN1 -> QKV -> attention -> out-proj+residual ->
LN2 -> GELU MLP -> residual) on its [1024, 1024] slice, fully on-chip.

v3 highlights over the bf16 baseline:
 - QKV, out-proj and attn@V matmuls run in fp8e4 with DoubleRow perf mode
   (2 contraction rows per PE pass). Softmax probs are produced in fp8
   directly by the Exp activation with an offset of -2 (cancelled by the
   self-consistent denominator from a ones-column appended to V).
 - Attention scores use PE row-tiling: head pairs occupy rows 0-63 /
   64-127 of the array concurrently (DH=64 contraction), doubling score
   throughput.
 - FFN stays bf16 (fp8 fails the 2e-2 tolerance) but weights are kept
   stationary for both token halves (2 matmuls per LDWEIGHTS).
 - Residual stream carried in fp32r; LN gammas/betas folded into weights
   and biases on the host. LN (x - mu) runs on the PE (rank-1 -mu + identity
   accumulate), rstd broadcast via GpSimd, so normalize is one DVE mul.
 - LN2 statistics are interleaved into the out-proj loop (per-chunk), V for
   heads 8-15 is produced inside the attention window to fill PE idle gaps.

Layouts: activations feature-major ([features(partitions), tokens(free)]).
LN stats via ones-vector matmuls.
"""

import sys, os

for _p in ("/root/.axon_site", "/root/.axon_site/_ro/trn_rl_repo",
           "/root/.axon_site/_ro/pypackages"):
    if _p not in sys.path:
        sys.path.append(_p)

import numpy as np
from contextlib import ExitStack

import concourse.bass as bass
import concourse.bacc as bacc
import concourse.mybir as mybir
import concourse.tile as tile
from concourse.bass_utils import run_bass_kernel_spmd

F32 = mybir.dt.float32
F32R = mybir.dt.float32r
BF16 = mybir.dt.bfloat16
FP8 = mybir.dt.float8e4
NP_BF16 = np.dtype(mybir.dt.np(BF16))
NP_FP8 = np.dtype(mybir.dt.np(FP8))
AF = mybir.ActivationFunctionType
ALU = mybir.AluOpType
DR = mybir.MatmulPerfMode.DoubleRow

B, P, E, H, DH, MLP = 8, 1024, 1024, 16, 64, 4096
SCALE = DH ** -0.5
NCORES = 8
EC = E // 128        # 8 feature chunks
TC = P // 128        # 8 token chunks
TN = P // 512        # 2 token halves
MC = MLP // 128      # 32 mlp chunks
HPAIRS = H // 2      # 8 head pairs
EXP_OFF = -2.0       # exp(s*SCALE - 2): keeps fp8 probs < 240 (self-cancelling)
VW = DH + 2          # v_sb row stride (64 v + 1 ones + 1 pad); slices use DH+1


def round_fp32r(x):
    b = np.ascontiguousarray(x, dtype=np.float32).view(np.uint32)
    b = ((b.astype(np.uint64) + 0x800) & 0xFFFFF000).astype(np.uint32)
    return b.view(np.float32)


def to_fp8(x):
    return np.clip(np.asarray(x, np.float32), -240.0, 240.0).astype(NP_FP8)


def build_program(dbg=False):
    nc = bacc.Bacc("TRN2", target_bir_lowering=False, debug=False,
                   num_devices=NCORES)

    xT_d = nc.dram_tensor("xT", [E, P], F32R, kind="ExternalInput").ap()
    wqkv8_d = nc.dram_tensor("wqkv8", [128, EC, 3 * E], FP8,
                             kind="ExternalInput").ap()
    wo8_d = nc.dram_tensor("wo8", [128, EC, E], FP8, kind="ExternalInput").ap()
    w1_d = nc.dram_tensor("w1", [128, MC, EC, 128], BF16,
                          kind="ExternalInput").ap()
    w2_d = nc.dram_tensor("w2", [128, MC, E], BF16, kind="ExternalInput").ap()
    bqkv_pm_d = nc.dram_tensor("bqkv_pm", [128, 16], F32,
                               kind="ExternalInput").ap()
    bv_row_d = nc.dram_tensor("bv_row", [1, E], F32R, kind="ExternalInput").ap()
    bo_pm_d = nc.dram_tensor("bo_pm", [128, EC], F32, kind="ExternalInput").ap()
    b1_pm_d = nc.dram_tensor("b1_pm", [128, MC], F32, kind="ExternalInput").ap()
    b2_pm_d = nc.dram_tensor("b2_pm", [128, EC], F32, kind="ExternalInput").ap()
    ones_row_d = nc.dram_tensor("ones_row", [1, 512], F32R,
                                kind="ExternalInput").ap()
    ones_col_d = nc.dram_tensor("ones_col", [128, 1], F32R,
                                kind="ExternalInput").ap()
    ones128_d = nc.dram_tensor("ones128", [128, 128], F32R,
                               kind="ExternalInput").ap()
    outT_d = nc.dram_tensor("outT", [E, P], F32, kind="ExternalOutput").ap()
    dbg_d = None
    if dbg:
        dbg_d = nc.dram_tensor("dbg", [8 * 1024, P], F32,
                               kind="ExternalOutput").ap()

    with tile.TileContext(nc) as tc, ExitStack() as ctx:
        const = ctx.enter_context(tc.tile_pool(name="const", bufs=1))
        rows = ctx.enter_context(tc.tile_pool(name="rows", bufs=2))
        scr = ctx.enter_context(tc.tile_pool(name="scr", bufs=2))

        def cload(shape, dt, dram, cname):
            t = const.tile(shape, dt, name=cname)
            nc.sync.dma_start(t[:], dram[:])
            return t

        ones_col = cload([128, 1], F32R, ones_col_d, "c_ones_col")
        ones_row = cload([1, 512], F32R, ones_row_d, "c_ones_row")
        ones128_r = cload([128, 128], F32R, ones128_d, "c_ones128_r")
        ones128_bf = const.tile([128, 128], BF16, name="c_ones128_bf")
        nc.vector.memset(ones128_bf[:], 1.0)

        # ---- load xT (f32r, feature-major) ----
        xTp = tc.alloc_tile_pool(name="xTp", bufs=1, side="right")
        xT = xTp.tile([128, EC, P], F32R, tag="xT", name="xT_sb")
        for c in range(EC):
            nc.sync.dma_start(xT[:, c, :], xT_d[c * 128:(c + 1) * 128, :])

        bqkv_pm = cload([128, 16], F32, bqkv_pm_d, "c_bqkv_pm")
        bv_row = cload([1, E], F32R, bv_row_d, "c_bv_row")
        bo_pm = cload([128, EC], F32, bo_pm_d, "c_bo_pm")
        b1_pm = cload([128, MC], F32, b1_pm_d, "c_b1_pm")
        b2_pm = cload([128, EC], F32, b2_pm_d, "c_b2_pm")
        eps_sb = const.tile([1, 1], F32, name="c_eps")
        nc.vector.memset(eps_sb[:], 1e-5)
        negoff = const.tile([128, 1], F32, name="c_negoff")
        nc.vector.memset(negoff[:], EXP_OFF)

        # ---- qkv / wo weights (fp8, fully resident) ----
        wo8p = tc.alloc_tile_pool(name="wo8p", bufs=1, side="right")
        wo8 = wo8p.tile([128, EC, E], FP8, tag="wo8", name="wo8_sb")
        nc.sync.dma_start(wo8[:], wo8_d[:])
        wq8p = tc.alloc_tile_pool(name="wq8p", bufs=1)
        wqkv8 = wq8p.tile([128, EC, 3 * E], FP8, tag="wqkv8", name="wqkv8_sb")
        for g in range(4):
            nc.sync.dma_start(wqkv8[:, 2 * g:2 * g + 2, :],
                              wqkv8_d[:, 2 * g:2 * g + 2, :])

        psA = tc.alloc_tile_pool(name="psA", bufs=4, space="PSUM")

        def ps_tile(nm):
            return psA.tile([128, 2, 512], F32, tag="ps", name=nm)

        # ---- LN helpers --------------------------------------------------
        # stats tiles are [128,2,512] psum: bank0 = E*mu broadcast over all
        # partitions (all-ones stationary), bank1 = E*E[x^2] broadcast.
        def ln_rows(st, nm):
            """Returns rstd broadcast [128,512] f32 SBUF (via GpSimd)."""
            msq = rows.tile([1, 512], F32, tag="r", bufs=6, name=f"{nm}_msq")
            nc.scalar.activation(msq[:], st[:1, 1, :], AF.Copy, scale=1.0 / E)
            mu = rows.tile([1, 512], F32, tag="r", bufs=6, name=f"{nm}_mu")
            nc.scalar.activation(mu[:], st[:1, 0, :], AF.Copy, scale=1.0 / E)
            mu2 = rows.tile([1, 512], F32, tag="r", bufs=6, name=f"{nm}_mu2")
            nc.vector.tensor_mul(mu2[:], mu[:], mu[:])
            var = rows.tile([1, 512], F32, tag="r", bufs=6, name=f"{nm}_var")
            nc.vector.tensor_sub(var[:], msq[:], mu2[:])
            lv = rows.tile([1, 512], F32, tag="r", bufs=6, name=f"{nm}_lv")
            nc.scalar.activation(lv[:], var[:], AF.Ln, bias=eps_sb[:])
            rstd = rows.tile([1, 512], F32, tag="mu", bufs=4,
                             name=f"{nm}_rstd")
            nc.scalar.activation(rstd[:], lv[:], AF.Exp, scale=-0.5)
            rb = scr.tile([128, 512], F32, tag="rb", bufs=2, name=f"{nm}_rb")
            nc.gpsimd.partition_broadcast(rb[:], rstd[:])
            return rb

        def ln_normalize(src_t, out_t, out_nm, st_tiles, rb_tiles):
            """out = (src - mu) * rstd; mu comes broadcast in st bank 0
            (scaled by E), subtract fused into one STT per chunk."""
            for tn in range(TN):
                sl = slice(tn * 512, (tn + 1) * 512)
                for c in range(EC):
                    d = scr.tile([128, 512], F32R, tag="lnd", bufs=4,
                                 name=f"{out_nm}_d{tn}_{c}")
                    nc.vector.scalar_tensor_tensor(
                        d[:], st_tiles[tn][:, 0, :], -1.0 / E,
                        src_t[:, c, sl], op0=ALU.mult, op1=ALU.add)
                    nc.vector.tensor_mul(out_t[:, c, sl], d[:], rb_tiles[tn][:])

        # ======== LN1 (gamma/beta folded into wqkv on host) ========
        xnp = tc.alloc_tile_pool(name="xnp", bufs=1)
        xnT = xnp.tile([128, EC, P], FP8, tag="xnT", name="xnT_sb")

        sq_tiles = []
        for c in range(EC):
            sq = scr.tile([128, P], BF16, tag="sq", bufs=8, name=f"ln1_sq{c}")
            nc.scalar.activation(sq[:], xT[:, c, :], AF.Square)
            sq_tiles.append(sq)

        ln1_st, ln1_rb = [], []
        for tn in range(TN):
            sl = slice(tn * 512, (tn + 1) * 512)
            st = ps_tile(f"ln1_st{tn}")
            for c in range(EC):
                nc.tensor.matmul(st[:, 0, :], ones128_r[:], xT[:, c, sl],
                                 start=(c == 0), stop=(c == EC - 1))
            for c in range(EC):
                nc.tensor.matmul(st[:, 1, :], ones128_bf[:],
                                 sq_tiles[c][:, sl],
                                 start=(c == 0), stop=(c == EC - 1))
            ln1_st.append(st)
            ln1_rb.append(ln_rows(st, f"ln1_{tn}"))
        ln_normalize(xT, xnT, "xn1", ln1_st, ln1_rb)

        def dump_fm(src_t, row0):
            dpool = tc.alloc_tile_pool(name=f"dump{row0}", bufs=1)
            for c in range(EC):
                st = dpool.tile([128, P], F32, tag="dump", name=f"dmp{row0}_{c}")
                nc.scalar.activation(st[:], src_t[:, c, :], AF.Copy)
                nc.sync.dma_start(dbg_d[row0 + c * 128:row0 + (c + 1) * 128, :],
                                  st[:])
            dpool.release()

        if dbg:
            dump_fm(xnT, 0)

        # ======== QKV ========
        # v: token-major output (xn stationary, weights moving)
        vp = tc.alloc_tile_pool(name="vp", bufs=1)
        v_sb = vp.tile([128, TC, H, VW], FP8, tag="v", name="v_sb")
        nc.vector.memset(v_sb[:, :, :, DH], 1.0)      # softmax-denominator col
        nc.vector.memset(v_sb[:, :, :, DH + 1], 0.0)  # pad

        def emit_v(vg):
            for tcp in range(TC // 2):
                ps = ps_tile(f"v_ps{vg}_{tcp}")
                for b_ in range(2):
                    tcc = 2 * tcp + b_
                    nc.tensor.matmul(ps[:, b_, :], ones_row[:, :128],
                                     bv_row[:, vg * 512:(vg + 1) * 512],
                                     start=True, stop=False)
                    for p8 in range(4):
                        nc.tensor.matmul(
                            ps[:, b_, :],
                            xnT[:, 2 * p8:2 * p8 + 2,
                                tcc * 128:(tcc + 1) * 128],
                            wqkv8[:, 2 * p8:2 * p8 + 2,
                                  2 * E + vg * 512:2 * E + (vg + 1) * 512],
                            start=False, stop=(p8 == 3), perf_mode=DR)
                for b_ in range(2):
                    tcc = 2 * tcp + b_
                    nc.vector.tensor_copy(
                        v_sb[:, tcc, vg * 8:(vg + 1) * 8, 0:DH],
                        ps[:, b_, :].rearrange("p (h d) -> p h d", d=DH))

        emit_v(0)

        # q, k then attention, per head pair
        qkp = tc.alloc_tile_pool(name="qkp", bufs=1)
        qT = qkp.tile([128, EC, P], BF16, tag="qT", name="qT_sb")
        kT = qkp.tile([128, EC, P], BF16, tag="kT", name="kT_sb")
        otp = tc.alloc_tile_pool(name="otp", bufs=1, side="right")
        oT = otp.tile([128, EC, P], FP8, tag="oT", name="oT_sb")
        atp = tc.alloc_tile_pool(name="atp", bufs=1, side="right")

        for hp in range(HPAIRS):
            for dst, fb, bi in ((qT, hp * 128, hp), (kT, E + hp * 128, 8 + hp)):
                ps = ps_tile(f"qk_ps{hp}_{fb}")
                for p8 in range(4):
                    lhsT = wqkv8[:, 2 * p8:2 * p8 + 2, fb:fb + 128]
                    for tn in range(TN):
                        nc.tensor.matmul(
                            ps[:, tn, :], lhsT,
                            xnT[:, 2 * p8:2 * p8 + 2,
                                tn * 512:(tn + 1) * 512],
                            start=(p8 == 0), stop=(p8 == 3), perf_mode=DR)
                nc.vector.tensor_scalar_add(
                    dst[:, hp, :], ps[:].rearrange("p a b -> p (a b)"),
                    bqkv_pm[:, bi:bi + 1])

            # ---- attention for heads (2hp, 2hp+1) ----
            for i in range(TN):
                isl = slice(i * 512, (i + 1) * 512)
                aT = atp.tile([128, 2, TC, 512], FP8, tag="aT", bufs=2,
                              name=f"aT_{hp}_{i}")
                for j in range(TC):
                    sps = ps_tile(f"s_ps{hp}_{i}_{j}")
                    jsl = slice(j * 128, (j + 1) * 128)
                    nc.tensor.matmul(sps[:, 0, :], kT[0:DH, hp, jsl],
                                     qT[0:DH, hp, isl], start=True, stop=True,
                                     tile_position=(0, 0))
                    nc.tensor.matmul(sps[:, 1, :], kT[DH:128, hp, jsl],
                                     qT[DH:128, hp, isl], start=True, stop=True,
                                     tile_position=(64, 0))
                    nc.scalar.activation(aT[:, :, j, :], sps[:], AF.Exp,
                                         scale=SCALE, bias=negoff[:])
                if dbg and hp == 0 and i == 0:
                    dap = tc.alloc_tile_pool(name="dumpa", bufs=1)
                    for jj in range(4):
                        sa = dap.tile([128, P], F32, tag="da",
                                      name=f"dmpa_{jj}")
                        for u2 in range(2):
                            nc.scalar.activation(
                                sa[:, u2 * 512:(u2 + 1) * 512],
                                aT[:, 0, 2 * jj + u2, :], AF.Copy)
                        nc.sync.dma_start(
                            dbg_d[7168 + jj * 128:7168 + (jj + 1) * 128, :],
                            sa[:])
                    dap.release()
                for u in range(2):
                    h = 2 * hp + u
                    pb = u * DH
                    ops = psA.tile([128, 512], F32, tag="ps",
                                   name=f"o_ps{hp}_{i}_{u}")
                    for jj in range(TC // 2):
                        nc.tensor.matmul(
                            ops[0:DH + 1, :],
                            v_sb[:, 2 * jj:2 * jj + 2, h, 0:DH + 1],
                            aT[:, u, 2 * jj:2 * jj + 2, :],
                            start=(jj == 0), stop=(jj == TC // 2 - 1),
                            perf_mode=DR)
                    den_row = rows.tile([1, 512], F32, tag="den", bufs=2,
                                        name=f"den_{hp}_{i}_{u}")
                    nc.vector.tensor_copy(den_row[:], ops[DH:DH + 1, :])
                    rec_row = rows.tile([1, 512], F32, tag="rec", bufs=2,
                                        name=f"rec_{hp}_{i}_{u}")
                    nc.vector.reciprocal_approx_fast(rec_row[:], den_row[:])
                    rec_b = scr.tile([DH, 512], F32, tag="recb", bufs=2,
                                     name=f"recb_{hp}_{i}_{u}")
                    nc.gpsimd.partition_broadcast(rec_b[:], rec_row[:])
                    nc.vector.tensor_mul(oT[pb:pb + DH, hp, isl],
                                         ops[0:DH, :], rec_b[:])

            if hp == 1:
                emit_v(1)   # fills PE idle in the exp-paced window

        if dbg:
            dump_fm(qT, 1024)
            dump_fm(kT, 2048)
            dvp = tc.alloc_tile_pool(name="dumpv", bufs=1)
            for tcc in range(TC):
                st = dvp.tile([128, H * DH], F32, tag="dumpv",
                              name=f"dmpv_{tcc}")
                nc.vector.tensor_copy(
                    st[:].rearrange("p (h d) -> p h d", d=DH),
                    v_sb[:, tcc, :, 0:DH])
                nc.sync.dma_start(
                    dbg_d[3072 + tcc * 128:3072 + (tcc + 1) * 128, :], st[:])
            dvp.release()
            dump_fm(oT, 4096)

        atp.release()
        qkp.release()
        vp.release()
        xnp.release()
        wq8p.release()

        # ======== out-proj + residual (fp8 DR), LN2 stats interleaved ======
        x2p = tc.alloc_tile_pool(name="x2p", bufs=1)
        x2T = x2p.tile([128, EC, P], F32R, tag="x2T", name="x2T_sb")
        st2 = [ps_tile(f"ln2_st{tn}") for tn in range(TN)]
        sq2_tiles = [scr.tile([128, P], BF16, tag="sq", bufs=8,
                              name=f"ln2_sq{c}") for c in range(EC)]
        for fc in range(EC):
            ps = ps_tile(f"wo_ps{fc}")
            for p8 in range(4):
                lhsT = wo8[:, 2 * p8:2 * p8 + 2, fc * 128:(fc + 1) * 128]
                for tn in range(TN):
                    nc.tensor.matmul(
                        ps[:, tn, :], lhsT,
                        oT[:, 2 * p8:2 * p8 + 2, tn * 512:(tn + 1) * 512],
                        start=(p8 == 0), stop=(p8 == 3), perf_mode=DR)
            nc.vector.scalar_tensor_tensor(
                x2T[:, fc, :], ps[:].rearrange("p a b -> p (a b)"),
                bo_pm[:, fc:fc + 1], xT[:, fc, :], op0=ALU.add, op1=ALU.add)
            # LN2 stats for this chunk (accumulate across the wo loop)
            nc.scalar.activation(sq2_tiles[fc][:], x2T[:, fc, :], AF.Square)
            for tn in range(TN):
                sl = slice(tn * 512, (tn + 1) * 512)
                nc.tensor.matmul(st2[tn][:, 0, :], ones128_r[:],
                                 x2T[:, fc, sl],
                                 start=(fc == 0), stop=(fc == EC - 1))
                nc.tensor.matmul(st2[tn][:, 1, :], ones128_bf[:],
                                 sq2_tiles[fc][:, sl],
                                 start=(fc == 0), stop=(fc == EC - 1))

        if dbg:
            dump_fm(x2T, 5120)

        otp.release()
        wo8p.release()
        xTp.release()

        # ======== LN2 rows + normalize (gamma/beta folded into w1/b1) ======
        xn2p = tc.alloc_tile_pool(name="xn2p", bufs=1)
        xn2T = xn2p.tile([128, EC, P], BF16, tag="xn2T", name="xn2T_sb")
        ln2_rb = [ln_rows(st2[tn], f"ln2_{tn}") for tn in range(TN)]
        ln_normalize(x2T, xn2T, "xn2", st2, ln2_rb)

        if dbg:
            dump_fm(xn2T, 6144)

        psA.release()

        # ======== FFN1 (bf16; 2 matmuls per LDWEIGHTS via token halves) ====
        w1p = tc.alloc_tile_pool(name="w1p", bufs=4)
        hp_pool = tc.alloc_tile_pool(name="hp", bufs=1, side="right")
        hT = hp_pool.tile([128, MC, P], BF16, tag="hT", name="hT_sb")
        psF = tc.alloc_tile_pool(name="psF", bufs=2, space="PSUM")

        for mc in range(MC):
            w1t = w1p.tile([128, EC, 128], BF16, tag="w1t", name=f"w1t_{mc}")
            nc.sync.dma_start(w1t[:], w1_d[:, mc, :, :])
            ps = psF.tile([128, 2, 512], F32, tag="f", name=f"h_ps{mc}")
            for ec in range(EC):
                lhsT = w1t[:, ec, :]
                for tn in range(TN):
                    nc.tensor.matmul(ps[:, tn, :], lhsT,
                                     xn2T[:, ec, tn * 512:(tn + 1) * 512],
                                     start=(ec == 0), stop=(ec == EC - 1))
            nc.scalar.activation(hT[:, mc, :],
                                 ps[:].rearrange("p a b -> p (a b)"),
                                 AF.Gelu, bias=b1_pm[:, mc:mc + 1])

        psF.release()

        # ======== FFN2 (bf16; 4 output chunks accumulate per fg pass) ======
        w1p.release()
        xn2p.release()
        w2p = tc.alloc_tile_pool(name="w2p", bufs=8)
        ogp = tc.alloc_tile_pool(name="ogp", bufs=2)
        psG = tc.alloc_tile_pool(name="psG", bufs=4, space="PSUM")

        for fg in range(2):
            ps2 = [psG.tile([128, 2, 512], F32, tag="g", name=f"o_ps{fg}_{fc}")
                   for fc in range(4)]
            for mc in range(MC):
                w2t = w2p.tile([128, 512], BF16, tag="w2t",
                               name=f"w2t_{fg}_{mc}")
                nc.sync.dma_start(w2t[:],
                                  w2_d[:, mc, fg * 512:(fg + 1) * 512])
                for fc in range(4):
                    lhsT = w2t[:, fc * 128:(fc + 1) * 128]
                    for tn in range(TN):
                        nc.tensor.matmul(
                            ps2[fc][:, tn, :], lhsT,
                            hT[:, mc, tn * 512:(tn + 1) * 512],
                            start=(mc == 0), stop=(mc == MC - 1))
            for fc in range(4):
                f = fg * 4 + fc
                og = ogp.tile([128, P], F32, tag="og", name=f"og_{f}")
                nc.vector.scalar_tensor_tensor(
                    og[:], ps2[fc][:].rearrange("p a b -> p (a b)"),
                    b2_pm[:, f:f + 1], x2T[:, f, :], op0=ALU.add, op1=ALU.add)
                nc.sync.dma_start(outT_d[f * 128:(f + 1) * 128, :], og[:])

        ogp.release()
        w2p.release()
        x2p.release()
        hp_pool.release()
        psG.release()
    return nc


def prep_inputs(x, ln1_g, ln1_b, wqkv, bqkv, wo, bo, ln2_g, ln2_b, w1, b1,
                w2, b2):
    """Host-side prep: shard x over batch, transpose feature-major, fold LN
    gammas/betas into the adjacent weights, quantize/cast, reorder weight
    blocks for chunked stationary loads."""
    ln1_g = np.asarray(ln1_g, np.float32)
    ln1_b = np.asarray(ln1_b, np.float32)
    ln2_g = np.asarray(ln2_g, np.float32)
    ln2_b = np.asarray(ln2_b, np.float32)
    wqkv = np.asarray(wqkv, np.float32)
    bqkv = np.asarray(bqkv, np.float32)
    wo_ = np.asarray(wo, np.float32)
    w1_ = np.asarray(w1, np.float32)
    w2_ = np.asarray(w2, np.float32)
    b1_ = np.asarray(b1, np.float32)

    # fold LN1 gamma into wqkv rows, beta into bqkv
    wqkv_f = wqkv * ln1_g[:, None]
    bqkv_f = bqkv + ln1_b @ wqkv
    # fold LN2 gamma into w1 rows, beta into b1
    w1_f = w1_ * ln2_g[:, None]
    b1_f = b1_ + ln2_b @ w1_

    def blk(w):  # [E, F] -> [128, EC_rows, F]
        Erows = w.shape[0]
        return np.ascontiguousarray(
            w.reshape(Erows // 128, 128, -1).transpose(1, 0, 2))

    def pm(vec, nchunks):
        return np.ascontiguousarray(
            np.asarray(vec, np.float32).reshape(nchunks, 128).T)

    w1_blk = blk(w1_f)                                  # [128, 8, 4096]
    w1_r = np.ascontiguousarray(                        # [128, 32(mc), 8, 128]
        w1_blk.reshape(128, EC, MC, 128).transpose(0, 2, 1, 3))

    shared = dict(
        wqkv8=to_fp8(blk(wqkv_f)),
        wo8=to_fp8(blk(wo_)),
        w1=w1_r.astype(NP_BF16),
        w2=blk(w2_).astype(NP_BF16),                    # [128, 32(mc), 1024]
        bqkv_pm=pm(bqkv_f[:2 * E], 16),
        bv_row=round_fp32r(bqkv_f[2 * E:].reshape(1, E)),
        bo_pm=pm(np.asarray(bo, np.float32), EC),
        b1_pm=pm(b1_f, MC),
        b2_pm=pm(np.asarray(b2, np.float32), EC),
        ones_row=np.ones((1, 512), np.float32),
        ones_col=np.ones((128, 1), np.float32),
        ones128=np.ones((128, 128), np.float32),
    )
    x = np.asarray(x, np.float32)
    in_maps = []
    for b in range(B):
        m = dict(shared)
        m["xT"] = round_fp32r(np.ascontiguousarray(x[b, :, :E].T))
        in_maps.append(m)
    return in_maps


_CACHE = {}


def run_on_hw(inputs, stage="full", trace=False, **trace_kw):
    key = stage
    if key not in _CACHE:
        nc = build_program(dbg=(stage == "dbg"))
        nc.compile()
        _CACHE[key] = nc
    nc = _CACHE[key]
    in_maps = prep_inputs(**inputs)
    res = run_bass_kernel_spmd(nc, in_maps, list(range(NCORES)), trace=trace,
                               **trace_kw)
    return res


def kernel(**inputs) -> np.ndarray:
    res = run_on_hw(inputs, stage="full", trace=False)
    out = np.zeros((B, P, E + 1), np.float32)
    for b in range(B):
        out[b, :, :E] = res.results[b]["outT"].T
    return out
